# revision 36
# baseline (speedup 1.0000x reference)
"""NT-Xent contrastive loss kernel for 8 Trainium2 NeuronCores.

Reference computation (N=8192, D=512, tau=0.5):
    zl = l2norm_rows(left); zr = l2norm_rows(right)
    refl    = exp(zl @ zl.T / tau)
    between = exp(zl @ zr.T / tau)
    denom   = refl.sum(1) + between.sum(1) - diag(refl)
    loss    = -log(diag(between) / denom)

Fused per-row form (diag(refl) == e^2 since rows of zl are unit):
    loss[m] = log( S_l[m] + S_r[m] - e^2 ) - 2 * (zl_m . zr_m)
with S_x[m] = sum_n exp(2 * zl_m . zx_n).

Sharding: data-parallel over rows; core c owns rows [c*1024, (c+1)*1024).
The host pre-normalizes rows (fp32), scales by 16 (keeps fp8e4m3 out of
subnormals; the 1/256 un-scale folds into the exp constants), casts to
fp8e4m3, and ships both tensors column-ROLLED so the core's own 1024
columns come first, chunked K-major as [P, KCH, W] slabs.

refl symmetry: the full refl matrix is symmetric, so each off-diagonal
1024x1024 block is computed ONCE globally.  In rolled-column offsets,
core c computes refl offsets 0,1 (chunk c0), 2,3 (chunk c1) and 4
(chunk c2, first half -- offset-4 pairs are computed by both ends, no
export), plus all 4 between chunks: 52 tile-equivalents instead of 64.
For the export offsets 1-3, the cell (m,n) also credits row n of the
partner core: the ACT exp for chunks c0/c1 lands in SBUF bf16, the Pool
engine (otherwise idle; it can ONLY reduce along the partition axis)
column-sums each [128, cols] slab into per-m-tile strips, and the
strips ship out as a third kernel output.  The host -- which already
does the O(N*D) normalize -- performs the O(N) cross-core credit sum
and the final log: no on-device collective, no transpose games.

On device each core runs fp8 DoubleRow matmuls (K=256/instr) of its
1024-row lhsT block against each chunk into [128,W] PSUM tiles, then
row-sum-exps each tile: most on ACT (exp with accum_out), a tuned
subset of non-export tiles on DVE via a Schraudolph exp (affine ->
int32 convert, then row-sum over the bitcast-float view).  The
between-diagonal is snapshotted off the raw own-r PSUM by an ACT Copy,
then masked+reduced on DVE from SBUF.

HW-measured op costs (microbench.py, repeat-slope): ACT exp [128,2048]
+accum ~1.9-2.1us; DVE affine+reduce pair ~3.4-4.2us (int16 variant is
SLOWER); one 8-matmul fp8-DR tile ~1.89us, ~90%% of the fp8 roofline.
PE busy: 64 tiles ~121us before the symmetry cut, ~99us after.
nc.vector.tensor_tensor_reduce with a PSUM input crashes real TRN2
(NRT_EXEC_UNIT_UNRECOVERABLE) -- do not reintroduce it.
"""

import numpy as np
import ml_dtypes
from contextlib import ExitStack

import concourse.bass as bass
import concourse.tile as tile
from concourse import bacc, mybir
from concourse.bass import ds, ts
from concourse.bass_utils import run_bass_kernel_spmd
from concourse.masks import make_identity

P = 128          # partitions
D = 512          # feature dim
N = 8192         # rows
NCORES = 8
BLK = N // NCORES          # 1024 rows per core
KCH = D // P               # 4 k-chunks of 128
MT = BLK // P              # 8 m-tiles per core
W = 2048                   # columns per full chunk
NJ = N // W                # 4 chunks per tensor
E2 = float(np.exp(2.0))

# device chunk schedule: c0, c1 (refl, exported), c2 first half (offset 4,
# both ends), then the 4 between chunks; r0 carries the between-diagonal.
CHUNKS = ["c0", "c1", "c2h", "r0", "r1", "r2", "r3"]
NCHUNK = len(CHUNKS)
J_DIAG = 3                 # r0
HALF = {"c2h"}             # 1024-wide tiles
EXPORT = {"c0", "c1"}      # exp -> SBUF bf16 + Pool colsum strips

SC = 16.0                  # host-side fp8 scale; dot products come out x256
INV = 1.0 / (SC * SC)
# Schraudolph exp(2s) from x = 256*s: bits = round(x*SA + SB), bitcast f32.
SA = float(2.0 * np.log2(np.e) * (1 << 23) * INV)
SB = float((1 << 23) * (127.0 - 0.03))

F32 = mybir.dt.float32
I32 = mybir.dt.int32
BF16 = mybir.dt.bfloat16
FP8 = mybir.dt.float8e4
AF = mybir.ActivationFunctionType
OP = mybir.AluOpType
DR = mybir.MatmulPerfMode.DoubleRow

# Tiles handed to the DVE Schraudolph path instead of ACT (j, m); only
# non-export, non-diagonal chunks are eligible (export tiles need their
# exp VALUES in SBUF for the Pool colsum; the diag chunk rides ACT).
DVE_COUNT = 12
REPEAT = 1                 # >1 only for slope benching (test-side)

_CACHE = {}


def _dve_tiles():
    elig = [
        (j, m)
        for j, name in enumerate(CHUNKS)
        if name not in EXPORT and j != J_DIAG
        for m in range(MT)
    ]
    n = min(DVE_COUNT, len(elig))
    if n <= 0:
        return set()
    step = len(elig) / n
    return {elig[int((i + 0.5) * step)] for i in range(n)}


def _body(ctx, tc, lch, rch, l2h, srow_out, bd_out, eblk_out):
    nc = tc.nc

    const_pool = ctx.enter_context(tc.tile_pool(name="const", bufs=1))
    persist = ctx.enter_context(tc.tile_pool(name="persist", bufs=1))
    zn_pool = ctx.enter_context(tc.tile_pool(name="zn", bufs=4))
    znh_pool = ctx.enter_context(tc.tile_pool(name="znh", bufs=1))
    i32_pool = ctx.enter_context(tc.tile_pool(name="i32", bufs=2))
    small = ctx.enter_context(tc.tile_pool(name="small", bufs=2))

    psum = ctx.enter_context(tc.tile_pool(name="ps", bufs=2, space="PSUM"))

    ident = const_pool.tile([P, P], F32, tag="ident")
    make_identity(nc, ident[:])
    # dummy first ACT instruction so the act-table load runs at t~0
    warm_in = const_pool.tile([P, 1], F32, tag="warm_in")
    nc.gpsimd.memset(warm_in[:], 0.0)
    warm = const_pool.tile([P, 1], F32, tag="warm")
    nc.scalar.activation(warm[:], warm_in[:], AF.Exp)

    zn_own = persist.tile([P, KCH, W], FP8, tag="zn_own")   # chunk c0
    rowsums = persist.tile([P, MT, NCHUNK], F32, tag="rowsums")
    bd = persist.tile([P, MT], F32, tag="bd")
    exp_buf = persist.tile([P, 2, MT, W], BF16, tag="exp_buf")

    zns = {}

    def dma_all():
        # c0 gates the pipeline: column-group slices (512B descriptor
        # lines) land group 0 -- the first rhs group and the m<4 lhsT
        # blocks -- after ~1/4 of a chunk time.
        for g in range(4):
            eng = nc.sync if g % 2 == 0 else nc.gpsimd
            eng.dma_start(
                zn_own[:, :, ds(g * 512, 512)], lch[0, :, :, ds(g * 512, 512)]
            )
        zns[0] = zn_own
        engines = [nc.sync, nc.gpsimd]
        # c1 is the second tile consumed; split it across both queues so it
        # lands right behind c0 instead of serializing after it.
        c1t = zn_pool.tile([P, KCH, W], FP8, tag="zn")
        nc.sync.dma_start(c1t[:, :, ds(0, 1024)], lch[1, :, :, ds(0, 1024)])
        nc.gpsimd.dma_start(c1t[:, :, ds(1024, 1024)], lch[1, :, :, ds(1024, 1024)])
        zns[1] = c1t
        srcs = {
            "c2h": l2h,
            "r0": rch[0, :, :, :],
            "r1": rch[1, :, :, :],
            "r2": rch[2, :, :, :],
            "r3": rch[3, :, :, :],
        }
        for j in range(2, NCHUNK):
            name = CHUNKS[j]
            if name in HALF:
                t = znh_pool.tile([P, KCH, W // 2], FP8, tag="znh")
            else:
                t = zn_pool.tile([P, KCH, W], FP8, tag="zn")
            engines[j % 2].dma_start(t[:, :, :], srcs[name])
            zns[j] = t

    dve_tiles = _dve_tiles()

    def main_stage(j):
        name = CHUNKS[j]
        zn = zns.pop(j)
        w = W // 2 if name in HALF else W
        ng = w // 512
        for m in range(MT):
            ps = psum.tile([P, W], F32, tag="act")
            for g in range(ng):
                for i in range(KCH // 2):
                    nc.tensor.matmul(
                        ps[:, ds(g * 512, 512)],
                        zn_own[:, ds(2 * i, 2), ts(m, P)],
                        zn[:, ds(2 * i, 2), ds(g * 512, 512)],
                        start=(i == 0),
                        stop=(i == KCH // 2 - 1),
                        perf_mode=DR,
                    )
            if j == J_DIAG:
                # own-r chunk: raw diagonal block IS the between-diag;
                # snapshot on ACT before the in-place exp, reduce on DVE.
                dcp = small.tile([P, P], F32, tag="dcp")
                nc.scalar.activation(dcp[:], ps[:, ds(m * P, P)], AF.Copy)
                dtmp = small.tile([P, P], F32, tag="dtmp")
                nc.vector.scalar_tensor_tensor(
                    out=dtmp[:], in0=dcp[:], scalar=INV, in1=ident[:],
                    op0=OP.mult, op1=OP.mult,
                )
                nc.vector.tensor_reduce(
                    bd[:, ts(m, 1)], dtmp[:],
                    axis=mybir.AxisListType.X, op=OP.add,
                )
            if name in EXPORT:
                # exp values land in SBUF bf16 for the Pool colsum
                eb = exp_buf[:, j, m, :]
                nc.scalar.activation(
                    eb, ps[:], AF.Exp, scale=2.0 * INV,
                    accum_out=rowsums[:, m, ds(j, 1)],
                )
                # ship the exp'd slab to DRAM (bf16, ~17us total overlapped
                # on the idle Pool DMA queue); the HOST does the column
                # sums -- the Pool C-axis tensor_reduce measures ~200us per
                # [128,2048] slab on real HW (Q7 software loop), 100x the
                # cost-model price, and PE ones-matmul colsums would eat
                # the PSUM banks the matmul pipeline needs.
                if name == "c0":
                    nc.gpsimd.dma_start(
                        eblk_out[m, :, ds(0, 1024)],
                        exp_buf[:, j, m, ds(1024, 1024)],
                    )
                else:
                    nc.gpsimd.dma_start(
                        eblk_out[m, :, ds(1024, 2048)], exp_buf[:, j, m, :]
                    )
            elif (j, m) in dve_tiles:
                t32 = i32_pool.tile([P, W], I32, tag="t32")
                nc.vector.tensor_scalar(
                    out=t32[:, ds(0, w)], in0=ps[:, ds(0, w)],
                    scalar1=SA, scalar2=SB, op0=OP.mult, op1=OP.add,
                )
                nc.vector.tensor_reduce(
                    rowsums[:, m, ds(j, 1)], t32[:, ds(0, w)].bitcast(F32),
                    axis=mybir.AxisListType.X, op=OP.add,
                )
            else:
                # exp output discarded (only accum_out matters): write back
                # into the same PSUM tile.
                nc.scalar.activation(
                    ps[:, ds(0, w)], ps[:, ds(0, w)], AF.Exp, scale=2.0 * INV,
                    accum_out=rowsums[:, m, ds(j, 1)],
                )

    dma_all()
    for j in range(NCHUNK):
        main_stage(j)

    # ---- outputs: partial row-sums, between-diag, export strips ----------
    # (the final log and the cross-core strip credits happen on the host)
    s_all = small.tile([P, MT], F32, tag="s_all")
    nc.vector.tensor_reduce(
        s_all[:], rowsums[:], axis=mybir.AxisListType.X, op=OP.add
    )
    nc.sync.dma_start(srow_out[:, :], s_all[:])
    nc.sync.dma_start(bd_out[:, :], bd[:])


class _pin_act_table:
    """During compile, present activation tables where Exp/Ln appear ONLY in
    the combined natural_log_exp table, so the table-load pass emits a single
    hoisted load.  Restored immediately after compile."""

    COMBINED = "natural_log_exp_and_others"

    def __enter__(self):
        import concourse.bacc as bacc_mod
        self._mod = bacc_mod
        self._orig = bacc_mod.get_activation_tables

        orig = self._orig
        combined = self.COMBINED

        def patched(arch):
            tabs = orig(arch)
            if combined not in tabs:
                return tabs
            pin = {AF.Exp, AF.Ln}
            out = {}
            for name, s in tabs.items():
                out[name] = set(s) if name == combined else set(s) - pin
            return out

        bacc_mod.get_activation_tables = patched
        return self

    def __exit__(self, *exc):
        self._mod.get_activation_tables = self._orig
        return False


def _build():
    nc = bacc.Bacc("TRN2", target_bir_lowering=False, debug=False, num_devices=NCORES)
    lch = nc.dram_tensor("lch", [2, P, KCH, W], FP8, kind="ExternalInput").ap()
    l2h = nc.dram_tensor("l2h", [P, KCH, W // 2], FP8, kind="ExternalInput").ap()
    rch = nc.dram_tensor("rch", [NJ, P, KCH, W], FP8, kind="ExternalInput").ap()
    srow = nc.dram_tensor("srow", [P, MT], F32, kind="ExternalOutput").ap()
    bdo = nc.dram_tensor("bd", [P, MT], F32, kind="ExternalOutput").ap()
    eblk = nc.dram_tensor(
        "eblk", [MT, P, 3 * 1024], BF16, kind="ExternalOutput"
    ).ap()
    with tile.TileContext(nc) as tc:
        for _ in range(REPEAT):
            with ExitStack() as ctx:
                _body(ctx, tc, lch, rch, l2h, srow, bdo, eblk)
    with _pin_act_table():
        nc.compile()
    return nc


def _get_nc():
    key = (REPEAT, DVE_COUNT)
    if key not in _CACHE:
        _CACHE[key] = _build()
    return _CACHE[key]


def _chunked(xT, c, nj):
    """xT: [KCH, P, N] fp8 K-major. Returns [nj, P, KCH, W] rolled so core
    c's own columns come first."""
    r = np.roll(xT, -c * BLK, axis=2)
    out = np.empty((nj, P, KCH, W), dtype=xT.dtype)
    for j in range(nj):
        out[j] = r[:, :, j * W:(j + 1) * W].transpose(1, 0, 2)
    return out


def _in_maps(left, right):
    f8 = ml_dtypes.float8_e4m3
    left = np.asarray(left, dtype=np.float32)
    right = np.asarray(right, dtype=np.float32)

    def prep(x):
        n = np.sqrt((x * x).sum(1, keepdims=True))
        z = x / np.maximum(n, 1e-12)
        return np.ascontiguousarray((z * SC).T).astype(f8).reshape(KCH, P, N)

    lT, rT = prep(left), prep(right)
    maps = []
    for c in range(NCORES):
        lall = _chunked(lT, c, 3)        # chunks c0, c1, c2 (full)
        maps.append({
            "lch": np.ascontiguousarray(lall[:2]),
            "l2h": np.ascontiguousarray(lall[2, :, :, :W // 2]),
            "rch": _chunked(rT, c, NJ),
        })
    return maps


def _gather(results):
    # srow/bd dram tiles are [128 partitions, 8 m-tiles]; row m = t*128 + p
    S = np.concatenate(
        [np.asarray(r["srow"], dtype=np.float64).T.reshape(-1) for r in results]
    )
    bd = np.concatenate(
        [np.asarray(r["bd"], dtype=np.float64).T.reshape(-1) for r in results]
    )
    # cross-core symmetric credits: core c's exported exp'd blocks for
    # offset d are column-summed here (bf16 -> f32 via bit widening) --
    # credit for rows of core c+d.
    for c, r in enumerate(results):
        eb = np.asarray(r["eblk"])
        f = (eb.view(np.uint16).astype(np.uint32) << 16).view(np.float32)
        st = f.sum(axis=(0, 1), dtype=np.float64)  # [3072] colsums
        for d in (1, 2, 3):
            tgt = (c + d) % NCORES
            S[tgt * BLK:(tgt + 1) * BLK] += st[(d - 1) * 1024: d * 1024]
    loss = np.log(S - E2) - 2.0 * bd
    return loss.astype(np.float32)


def run_traced(left, right):
    """Run with NTFF profiling; returns (loss, exec_time_ns)."""
    res = run_bass_kernel_spmd(
        _get_nc(), _in_maps(left, right), list(range(NCORES)), trace=True
    )
    return _gather(res.results), res.exec_time_ns


def kernel(left, right):
    res = run_bass_kernel_spmd(
        _get_nc(), _in_maps(left, right), list(range(NCORES))
    )
    return _gather(res.results)


# revision 39
# speedup vs baseline: 1.0173x; 1.0173x over previous
"""NT-Xent contrastive loss kernel for 8 Trainium2 NeuronCores.

Reference computation (N=8192, D=512, tau=0.5):
    zl = l2norm_rows(left); zr = l2norm_rows(right)
    refl    = exp(zl @ zl.T / tau)
    between = exp(zl @ zr.T / tau)
    denom   = refl.sum(1) + between.sum(1) - diag(refl)
    loss    = -log(diag(between) / denom)

Fused per-row form (diag(refl) == e^2 since rows of zl are unit):
    loss[m] = log( S_l[m] + S_r[m] - e^2 ) - 2 * (zl_m . zr_m)
with S_x[m] = sum_n exp(2 * zl_m . zx_n).

Sharding: data-parallel over rows; core c owns rows [c*1024, (c+1)*1024).
The host pre-normalizes rows (fp32), scales by 16 (keeps fp8e4m3 out of
subnormals; the 1/256 un-scale folds into the exp constants), casts to
fp8e4m3, and ships both tensors column-ROLLED so the core's own 1024
columns come first, chunked K-major as [P, KCH, W] slabs.

refl symmetry: the full refl matrix is symmetric, so each off-diagonal
1024x1024 block is computed ONCE globally.  In rolled-column offsets,
core c computes refl offsets 0,1 (chunk c0), 2,3 (chunk c1) and 4
(chunk c2, first half -- offset-4 pairs are computed by both ends, no
export), plus all 4 between chunks: 52 tile-equivalents instead of 64.
For the export offsets 1-3, the cell (m,n) also credits row n of the
partner core: the ACT exp for chunks c0/c1 lands in SBUF bf16, the Pool
engine (otherwise idle; it can ONLY reduce along the partition axis)
column-sums each [128, cols] slab into per-m-tile strips, and the
strips ship out as a third kernel output.  The host -- which already
does the O(N*D) normalize -- performs the O(N) cross-core credit sum
and the final log: no on-device collective, no transpose games.

On device each core runs fp8 DoubleRow matmuls (K=256/instr) of its
1024-row lhsT block against each chunk into [128,W] PSUM tiles, then
row-sum-exps each tile: most on ACT (exp with accum_out), a tuned
subset of non-export tiles on DVE via a Schraudolph exp (affine ->
int32 convert, then row-sum over the bitcast-float view).  The
between-diagonal is snapshotted off the raw own-r PSUM by an ACT Copy,
then masked+reduced on DVE from SBUF.

HW-measured op costs (microbench.py, repeat-slope): ACT exp [128,2048]
+accum ~1.9-2.1us; DVE affine+reduce pair ~3.4-4.2us (int16 variant is
SLOWER); one 8-matmul fp8-DR tile ~1.89us, ~90%% of the fp8 roofline.
PE busy: 64 tiles ~121us before the symmetry cut, ~99us after.
nc.vector.tensor_tensor_reduce with a PSUM input crashes real TRN2
(NRT_EXEC_UNIT_UNRECOVERABLE) -- do not reintroduce it.
"""

import numpy as np
import ml_dtypes
from contextlib import ExitStack

import concourse.bass as bass
import concourse.tile as tile
from concourse import bacc, mybir
from concourse.bass import ds, ts
from concourse.bass_utils import run_bass_kernel_spmd
from concourse.masks import make_identity

P = 128          # partitions
D = 512          # feature dim
N = 8192         # rows
NCORES = 8
BLK = N // NCORES          # 1024 rows per core
KCH = D // P               # 4 k-chunks of 128
MT = BLK // P              # 8 m-tiles per core
W = 2048                   # columns per full chunk
NJ = N // W                # 4 chunks per tensor
E2 = float(np.exp(2.0))

# device chunk schedule: c0, c1 (refl, exported), c2 first half (offset 4,
# both ends), then the 4 between chunks; r0 carries the between-diagonal.
CHUNKS = ["c0", "c1", "c2h", "r0", "r1", "r2", "r3"]
NCHUNK = len(CHUNKS)
J_DIAG = 3                 # r0
HALF = {"c2h"}             # 1024-wide tiles
EXPORT = {"c0", "c1"}      # exp -> SBUF bf16 + Pool colsum strips

SC = 16.0                  # host-side fp8 scale; dot products come out x256
INV = 1.0 / (SC * SC)
# Schraudolph exp(2s) from x = 256*s: bits = round(x*SA + SB), bitcast f32.
SA = float(2.0 * np.log2(np.e) * (1 << 23) * INV)
SB = float((1 << 23) * (127.0 - 0.03))

F32 = mybir.dt.float32
I32 = mybir.dt.int32
BF16 = mybir.dt.bfloat16
FP8 = mybir.dt.float8e4
AF = mybir.ActivationFunctionType
OP = mybir.AluOpType
DR = mybir.MatmulPerfMode.DoubleRow

# Tiles handed to the DVE Schraudolph path instead of ACT (j, m); only
# non-export, non-diagonal chunks are eligible (export tiles need their
# exp VALUES in SBUF for the Pool colsum; the diag chunk rides ACT).
DVE_COUNT = 12
REPEAT = 1                 # >1 only for slope benching (test-side)

_CACHE = {}


def _dve_tiles():
    elig = [
        (j, m)
        for j, name in enumerate(CHUNKS)
        if name not in EXPORT and j != J_DIAG
        for m in range(MT)
    ]
    n = min(DVE_COUNT, len(elig))
    if n <= 0:
        return set()
    step = len(elig) / n
    return {elig[int((i + 0.5) * step)] for i in range(n)}


def _body(ctx, tc, lch, rch, l2h, srow_out, bd_out, eblk_out):
    nc = tc.nc

    const_pool = ctx.enter_context(tc.tile_pool(name="const", bufs=1))
    persist = ctx.enter_context(tc.tile_pool(name="persist", bufs=1))
    zn_pool = ctx.enter_context(tc.tile_pool(name="zn", bufs=4))
    znh_pool = ctx.enter_context(tc.tile_pool(name="znh", bufs=1))
    i32_pool = ctx.enter_context(tc.tile_pool(name="i32", bufs=2))
    small = ctx.enter_context(tc.tile_pool(name="small", bufs=2))

    psum = ctx.enter_context(tc.tile_pool(name="ps", bufs=2, space="PSUM"))

    ident = const_pool.tile([P, P], F32, tag="ident")
    make_identity(nc, ident[:])
    # dummy first ACT instruction so the act-table load runs at t~0
    warm_in = const_pool.tile([P, 1], F32, tag="warm_in")
    nc.gpsimd.memset(warm_in[:], 0.0)
    warm = const_pool.tile([P, 1], F32, tag="warm")
    nc.scalar.activation(warm[:], warm_in[:], AF.Exp)

    zn_own = persist.tile([P, KCH, W], FP8, tag="zn_own")   # chunk c0
    rowsums = persist.tile([P, MT, NCHUNK], F32, tag="rowsums")
    bd = persist.tile([P, MT], F32, tag="bd")
    # fp8 export: exp values lie in [e^-2, e^2] (well inside e4m3 range);
    # the ~3%% per-element quantization averages out in the 1024-row column
    # sums (~0.1%% on credits that are ~1/3 of the denominator).  Halves
    # both the SBUF footprint and the DRAM export volume vs bf16.
    exp_buf = persist.tile([P, 2, MT, W], FP8, tag="exp_buf")

    zns = {}

    def dma_all():
        # c0 gates the pipeline: column-group slices (512B descriptor
        # lines) land group 0 -- the first rhs group and the m<4 lhsT
        # blocks -- after ~1/4 of a chunk time.
        for g in range(4):
            eng = nc.sync if g % 2 == 0 else nc.gpsimd
            eng.dma_start(
                zn_own[:, :, ds(g * 512, 512)], lch[0, :, :, ds(g * 512, 512)]
            )
        zns[0] = zn_own
        engines = [nc.sync, nc.gpsimd]
        # c1 is the second tile consumed; split it across both queues so it
        # lands right behind c0 instead of serializing after it.
        c1t = zn_pool.tile([P, KCH, W], FP8, tag="zn")
        nc.sync.dma_start(c1t[:, :, ds(0, 1024)], lch[1, :, :, ds(0, 1024)])
        nc.gpsimd.dma_start(c1t[:, :, ds(1024, 1024)], lch[1, :, :, ds(1024, 1024)])
        zns[1] = c1t
        srcs = {
            "c2h": l2h,
            "r0": rch[0, :, :, :],
            "r1": rch[1, :, :, :],
            "r2": rch[2, :, :, :],
            "r3": rch[3, :, :, :],
        }
        for j in range(2, NCHUNK):
            name = CHUNKS[j]
            if name in HALF:
                t = znh_pool.tile([P, KCH, W // 2], FP8, tag="znh")
            else:
                t = zn_pool.tile([P, KCH, W], FP8, tag="zn")
            engines[j % 2].dma_start(t[:, :, :], srcs[name])
            zns[j] = t

    dve_tiles = _dve_tiles()

    def main_stage(j):
        name = CHUNKS[j]
        zn = zns.pop(j)
        w = W // 2 if name in HALF else W
        ng = w // 512
        for m in range(MT):
            ps = psum.tile([P, W], F32, tag="act")
            for g in range(ng):
                for i in range(KCH // 2):
                    nc.tensor.matmul(
                        ps[:, ds(g * 512, 512)],
                        zn_own[:, ds(2 * i, 2), ts(m, P)],
                        zn[:, ds(2 * i, 2), ds(g * 512, 512)],
                        start=(i == 0),
                        stop=(i == KCH // 2 - 1),
                        perf_mode=DR,
                    )
            if j == J_DIAG:
                # own-r chunk: raw diagonal block IS the between-diag;
                # snapshot on ACT before the in-place exp, reduce on DVE.
                dcp = small.tile([P, P], F32, tag="dcp")
                nc.scalar.activation(dcp[:], ps[:, ds(m * P, P)], AF.Copy)
                dtmp = small.tile([P, P], F32, tag="dtmp")
                nc.vector.scalar_tensor_tensor(
                    out=dtmp[:], in0=dcp[:], scalar=INV, in1=ident[:],
                    op0=OP.mult, op1=OP.mult,
                )
                nc.vector.tensor_reduce(
                    bd[:, ts(m, 1)], dtmp[:],
                    axis=mybir.AxisListType.X, op=OP.add,
                )
            if name in EXPORT:
                # exp values land in SBUF bf16 for the Pool colsum
                eb = exp_buf[:, j, m, :]
                nc.scalar.activation(
                    eb, ps[:], AF.Exp, scale=2.0 * INV,
                    accum_out=rowsums[:, m, ds(j, 1)],
                )
                # ship the exp'd slab to DRAM (bf16, ~17us total overlapped
                # on the idle Pool DMA queue); the HOST does the column
                # sums -- the Pool C-axis tensor_reduce measures ~200us per
                # [128,2048] slab on real HW (Q7 software loop), 100x the
                # cost-model price, and PE ones-matmul colsums would eat
                # the PSUM banks the matmul pipeline needs.
                if name == "c0":
                    nc.gpsimd.dma_start(
                        eblk_out[m, :, ds(0, 1024)],
                        exp_buf[:, j, m, ds(1024, 1024)],
                    )
                else:
                    nc.gpsimd.dma_start(
                        eblk_out[m, :, ds(1024, 2048)], exp_buf[:, j, m, :]
                    )
            elif (j, m) in dve_tiles:
                t32 = i32_pool.tile([P, W], I32, tag="t32")
                nc.vector.tensor_scalar(
                    out=t32[:, ds(0, w)], in0=ps[:, ds(0, w)],
                    scalar1=SA, scalar2=SB, op0=OP.mult, op1=OP.add,
                )
                nc.vector.tensor_reduce(
                    rowsums[:, m, ds(j, 1)], t32[:, ds(0, w)].bitcast(F32),
                    axis=mybir.AxisListType.X, op=OP.add,
                )
            else:
                # exp output discarded (only accum_out matters): write back
                # into the same PSUM tile.
                nc.scalar.activation(
                    ps[:, ds(0, w)], ps[:, ds(0, w)], AF.Exp, scale=2.0 * INV,
                    accum_out=rowsums[:, m, ds(j, 1)],
                )

    dma_all()
    for j in range(NCHUNK):
        main_stage(j)

    # ---- outputs: partial row-sums, between-diag, export strips ----------
    # (the final log and the cross-core strip credits happen on the host)
    s_all = small.tile([P, MT], F32, tag="s_all")
    nc.vector.tensor_reduce(
        s_all[:], rowsums[:], axis=mybir.AxisListType.X, op=OP.add
    )
    nc.sync.dma_start(srow_out[:, :], s_all[:])
    nc.sync.dma_start(bd_out[:, :], bd[:])


class _pin_act_table:
    """During compile, present activation tables where Exp/Ln appear ONLY in
    the combined natural_log_exp table, so the table-load pass emits a single
    hoisted load.  Restored immediately after compile."""

    COMBINED = "natural_log_exp_and_others"

    def __enter__(self):
        import concourse.bacc as bacc_mod
        self._mod = bacc_mod
        self._orig = bacc_mod.get_activation_tables

        orig = self._orig
        combined = self.COMBINED

        def patched(arch):
            tabs = orig(arch)
            if combined not in tabs:
                return tabs
            pin = {AF.Exp, AF.Ln}
            out = {}
            for name, s in tabs.items():
                out[name] = set(s) if name == combined else set(s) - pin
            return out

        bacc_mod.get_activation_tables = patched
        return self

    def __exit__(self, *exc):
        self._mod.get_activation_tables = self._orig
        return False


def _build():
    nc = bacc.Bacc("TRN2", target_bir_lowering=False, debug=False, num_devices=NCORES)
    lch = nc.dram_tensor("lch", [2, P, KCH, W], FP8, kind="ExternalInput").ap()
    l2h = nc.dram_tensor("l2h", [P, KCH, W // 2], FP8, kind="ExternalInput").ap()
    rch = nc.dram_tensor("rch", [NJ, P, KCH, W], FP8, kind="ExternalInput").ap()
    srow = nc.dram_tensor("srow", [P, MT], F32, kind="ExternalOutput").ap()
    bdo = nc.dram_tensor("bd", [P, MT], F32, kind="ExternalOutput").ap()
    eblk = nc.dram_tensor(
        "eblk", [MT, P, 3 * 1024], FP8, kind="ExternalOutput"
    ).ap()
    with tile.TileContext(nc) as tc:
        for _ in range(REPEAT):
            with ExitStack() as ctx:
                _body(ctx, tc, lch, rch, l2h, srow, bdo, eblk)
    with _pin_act_table():
        nc.compile()
    return nc


def _get_nc():
    key = (REPEAT, DVE_COUNT)
    if key not in _CACHE:
        _CACHE[key] = _build()
    return _CACHE[key]


def _chunked(xT, c, nj):
    """xT: [KCH, P, N] fp8 K-major. Returns [nj, P, KCH, W] rolled so core
    c's own columns come first."""
    r = np.roll(xT, -c * BLK, axis=2)
    out = np.empty((nj, P, KCH, W), dtype=xT.dtype)
    for j in range(nj):
        out[j] = r[:, :, j * W:(j + 1) * W].transpose(1, 0, 2)
    return out


def _in_maps(left, right):
    f8 = ml_dtypes.float8_e4m3
    left = np.asarray(left, dtype=np.float32)
    right = np.asarray(right, dtype=np.float32)

    def prep(x):
        n = np.sqrt((x * x).sum(1, keepdims=True))
        z = x / np.maximum(n, 1e-12)
        return np.ascontiguousarray((z * SC).T).astype(f8).reshape(KCH, P, N)

    lT, rT = prep(left), prep(right)
    maps = []
    for c in range(NCORES):
        lall = _chunked(lT, c, 3)        # chunks c0, c1, c2 (full)
        maps.append({
            "lch": np.ascontiguousarray(lall[:2]),
            "l2h": np.ascontiguousarray(lall[2, :, :, :W // 2]),
            "rch": _chunked(rT, c, NJ),
        })
    return maps


def _gather(results):
    # srow/bd dram tiles are [128 partitions, 8 m-tiles]; row m = t*128 + p
    S = np.concatenate(
        [np.asarray(r["srow"], dtype=np.float64).T.reshape(-1) for r in results]
    )
    bd = np.concatenate(
        [np.asarray(r["bd"], dtype=np.float64).T.reshape(-1) for r in results]
    )
    # cross-core symmetric credits: core c's exported exp'd blocks for
    # offset d are column-summed here (bf16 -> f32 via bit widening) --
    # credit for rows of core c+d.
    for c, r in enumerate(results):
        f = np.asarray(r["eblk"]).astype(np.float32)
        st = f.sum(axis=(0, 1), dtype=np.float64)  # [3072] colsums
        for d in (1, 2, 3):
            tgt = (c + d) % NCORES
            S[tgt * BLK:(tgt + 1) * BLK] += st[(d - 1) * 1024: d * 1024]
    loss = np.log(S - E2) - 2.0 * bd
    return loss.astype(np.float32)


def run_traced(left, right):
    """Run with NTFF profiling; returns (loss, exec_time_ns)."""
    res = run_bass_kernel_spmd(
        _get_nc(), _in_maps(left, right), list(range(NCORES)), trace=True
    )
    return _gather(res.results), res.exec_time_ns


def kernel(left, right):
    res = run_bass_kernel_spmd(
        _get_nc(), _in_maps(left, right), list(range(NCORES))
    )
    return _gather(res.results)
